# revision 10
# baseline (speedup 1.0000x reference)
"""AngleLoss (HANDS17 bone-angle loss) on 8 TRN2 NeuronCores.

Math (per batch element b, bone pair (i0, i1)):
    v1 = pred[b, i0, :2] - pred[b, i1, :2]
    v2 = gt[b, i0, :2]   - gt[b, i1, :2]
    t  = |v1 . v2| / (|v1| |v2|)
    loss = mean over (b, pair) of (1 - t)

Strategy: pure data parallel over the batch; each core streams its
65536-element shard (33 MB f32) through SBUF, which pins the roofline
at the ~358 GB/s per-core DMA rate (~96 us). The elementwise pipeline
is spread over three engines so every engine's true work stays under
that floor. Measured steady-state rates driving the assignment (ns per
max-AP element): DVE 2x contiguous bf16 0.55-0.60, interleaved-read
f32->bf16 cast 0.56, bf16 uv-pair subs 0.66, stride-2 pair add 1.17;
ACT ~0.9 for everything; Pool ~3.0 (otherwise idle).

  - f32->bf16 casts from the [c, j, xy] interleaved layout (packed 8B
    uv-pair reads, the fast pattern). The pred cast is split between
    ACT and DVE for load balance; gt cast on DVE. Casting (not direct
    f32 subs) also releases the input DMA buffers early, which keeps
    the DMA queues from being paced by compute.
  - 4 bone-pair subtract gathers (DVE 2-run bf16 operands); the
    HANDS17 pair list decomposes into four affine gathers.
  - prod = v1*v2 (DVE 2x contiguous).
  - sq = dc^2 on ACT with a TRANSPOSED [xy-outer] output, so the
    |v|^2 pair-reduction is a contiguous-half add (DVE 2x).
  - dot = stride-2 pair add (DVE), den = n1*n2 (Pool), e =
    exp(-0.5 ln(den+eps)) (ACT; Rsqrt is banned in bass), a = |dot|
    (ACT), t = a*e (Pool), ones-matmul batch reduction (PE -> PSUM).

Stage B of tile i-1 is emitted BEFORE stage A of tile i so each
in-order engine queue always has ready work at its head, and the input
pool is triple-buffered so the DMA runs ~3 tiles ahead. Tile sizes
ramp small -> 64 -> small to shorten the initial DMA wait and the
final drain.
"""
import sys

sys.path.insert(0, "/opt/trn_rl_repo")

from contextlib import ExitStack

import numpy as np

import concourse.bass as bass
import concourse.tile as tile
from concourse import mybir
from concourse.bass_utils import run_bass_kernel_spmd

B, J, DCOORD = 524288, 21, 3
NCORES = 8
P = 128                      # SBUF partitions
F = J * DCOORD               # 63 floats per batch element
NPAIR = 20

f32 = mybir.dt.float32
bf16 = mybir.dt.bfloat16
AF = mybir.ActivationFunctionType


def _split_excess_waits(nc, max_waits: int = 1) -> int:
    """The staged neuronxcc rejects instructions with more than one
    semaphore wait. Same-engine instructions run in order, so excess
    waits move onto preceding NoOps on the same engine."""
    n_split = 0
    for b in nc.m.functions[0].blocks:
        insts = b.instructions
        out = []
        changed = False
        for inst in insts:
            si = getattr(inst, "sync_info", None)
            waits = list(si.on_wait) if si is not None and si.on_wait else []
            if len(waits) > max_waits:
                extra, keep = waits[:-max_waits], waits[-max_waits:]
                while extra:
                    grp, extra = extra[:max_waits], extra[max_waits:]
                    nop = mybir.InstNoOp(
                        name=f"I-waitsplit-{n_split}", engine=inst.engine
                    )
                    nop.sync_info = mybir.SyncInfo(on_wait=grp, on_update=[])
                    out.append(nop)
                    n_split += 1
                inst.sync_info = mybir.SyncInfo(
                    on_wait=keep, on_update=list(si.on_update)
                )
                changed = True
            out.append(inst)
        if changed:
            insts[:] = out
    return n_split


def build_nc(tiles) -> bass.Bass:
    """One core's kernel. `tiles` is the list of per-tile batch counts C
    (batch elements per partition); total batch = P * sum(tiles)."""
    BL = P * sum(tiles)
    nc = bass.Bass()
    x_ext = nc.declare_dram_parameter("jt_uvd_pred", [BL, F], f32, isOutput=False)
    g_ext = nc.declare_dram_parameter("jt_uvd_gt", [BL, F], f32, isOutput=False)
    out_ext = nc.declare_dram_parameter("out", [1, 1], f32, isOutput=True)
    NFMAX = NPAIR * max(tiles)

    with tile.TileContext(nc) as tc, ExitStack() as ctx:
        ins_pool = ctx.enter_context(tc.tile_pool(name="ins", bufs=3))
        mid_pool = ctx.enter_context(tc.tile_pool(name="mid", bufs=2))
        small_pool = ctx.enter_context(tc.tile_pool(name="small", bufs=2))
        const_pool = ctx.enter_context(tc.tile_pool(name="const", bufs=1))
        psum_pool = ctx.enter_context(tc.tile_pool(name="psum", bufs=1, space="PSUM"))

        ones = const_pool.tile([P, 1], bf16)
        nc.vector.memset(ones[:], 1.0)
        # bf16-rounded bones can collide -> den=0; ln(den+eps) keeps those
        # pairs at t = 0*huge = 0 instead of NaN
        eps = const_pool.tile([P, 1], f32)
        nc.vector.memset(eps[:], 1e-30)

        # PSUM accumulators for the batch reduction, <=512 f32 per bank.
        # Zeroed up front so variable-size tiles can all accumulate with
        # start=False.
        psums = []
        off = 0
        while off < NFMAX:
            w = min(512, NFMAX - off)
            ps = psum_pool.tile([1, w], f32, name=f"ps{off}", tag=f"ps{off}")
            nc.vector.memset(ps[:], 0.0)
            psums.append((off, w, ps))
            off += w
        last_user = {}
        for i, C in enumerate(tiles):
            for k, (poff, w, ps) in enumerate(psums):
                if NPAIR * C > poff:
                    last_user[k] = i

        state = {}
        b0 = 0

        def emit_a(i):
            nonlocal b0
            C = tiles[i]
            FD = C * F
            rows = P * C
            xv = x_ext[b0 : b0 + rows, :].rearrange("(p c) f -> p (c f)", p=P)
            gv = g_ext[b0 : b0 + rows, :].rearrange("(p c) f -> p (c f)", p=P)
            b0 += rows

            xt = ins_pool.tile([P, FD], f32, tag="xin")
            gt = ins_pool.tile([P, FD], f32, tag="gin")
            nc.sync.dma_start(out=xt[:], in_=xv)
            nc.sync.dma_start(out=gt[:], in_=gv)

            # combined bf16 uv tile: rows 0:C pred, C:2C gt. [c][j][xy]
            # read pattern = packed 8B runs (fast). Pred cast split
            # ACT/DVE for engine balance; ins buffers free right here.
            u = mid_pool.tile([P, 2 * C, J, 2], bf16, tag="u")
            h = C // 2
            xs = xt[:].rearrange("p (c j k) -> p c j k", j=J, k=DCOORD)[:, :, :, 0:2]
            gs = gt[:].rearrange("p (c j k) -> p c j k", j=J, k=DCOORD)[:, :, :, 0:2]
            nc.scalar.activation(out=u[:, 0:h], in_=xs[:, 0:h], func=AF.Copy)
            nc.vector.tensor_copy(out=u[:, h:C], in_=xs[:, h:C])
            nc.vector.tensor_copy(out=u[:, C : 2 * C], in_=gs)

            # bone vectors dc[(t c), q, xy]: four affine gathers, every
            # operand a packed uv pair -> DVE 2x
            dc = mid_pool.tile([P, 2 * C, NPAIR, 2], bf16, tag="dc")
            root = u[:, :, 0:1, :].broadcast_to([P, 2 * C, 5, 2])
            subs = [
                (0, root, u[:, :, 1:6, :]),
                (5, u[:, :, 1:6, :], u[:, :, 6:19:3, :]),
                (10, u[:, :, 6:19:3, :], u[:, :, 7:20:3, :]),
                (15, u[:, :, 7:20:3, :], u[:, :, 8:21:3, :]),
            ]
            for s0, in0, in1 in subs:
                nc.vector.tensor_sub(out=dc[:, :, s0 : s0 + 5, :], in0=in0, in1=in1)

            # v1*v2, contiguous bf16 -> DVE 2x
            pr = mid_pool.tile([P, C, NPAIR, 2], bf16, tag="pr")
            nc.vector.tensor_mul(
                out=pr[:].rearrange("p c q k -> p (c q k)"),
                in0=dc[:, 0:C].rearrange("p c q k -> p (c q k)"),
                in1=dc[:, C : 2 * C].rearrange("p c q k -> p (c q k)"),
            )
            # squares on ACT with TRANSPOSED [xy-outer] output so the
            # pair-reduction below is contiguous
            s = mid_pool.tile([P, 2, 2 * C, NPAIR], bf16, tag="s")
            nc.scalar.activation(
                out=s[:], in_=dc[:].rearrange("p c q k -> p k c q"), func=AF.Square
            )
            state[i] = (C, pr, s)

        def emit_b(i):
            C, pr, s = state.pop(i)
            NF = NPAIR * C
            # dot = x-part + y-part (stride-2 halves of pr)
            dot = small_pool.tile([P, C, NPAIR], bf16, tag="dot")
            nc.vector.tensor_add(out=dot[:], in0=pr[:, :, :, 0], in1=pr[:, :, :, 1])
            # n[(t c), q] = |v|^2 per tensor: contiguous halves (DVE 2x)
            n = small_pool.tile([P, 2 * C, NPAIR], bf16, tag="n")
            nc.vector.tensor_add(
                out=n[:].rearrange("p c q -> p (c q)"),
                in0=s[:, 0].rearrange("p c q -> p (c q)"),
                in1=s[:, 1].rearrange("p c q -> p (c q)"),
            )
            # den = n1 * n2 on the otherwise idle Pool engine
            den = small_pool.tile([P, C, NPAIR], bf16, tag="den")
            nc.gpsimd.tensor_mul(
                out=den[:].rearrange("p c q -> p (c q)"),
                in0=n[:, 0:C].rearrange("p c q -> p (c q)"),
                in1=n[:, C : 2 * C].rearrange("p c q -> p (c q)"),
            )
            # e = 1/sqrt(den) = exp(-0.5*ln(den+eps)) on ACT (Rsqrt is
            # banned in bass for accuracy; Ln/Exp/Abs share one table set)
            lg = small_pool.tile([P, NF], bf16, tag="lg")
            nc.scalar.activation(
                out=lg[:],
                in_=den[:].rearrange("p c q -> p (c q)"),
                func=AF.Ln,
                bias=eps[:],
            )
            e = small_pool.tile([P, NF], bf16, tag="e")
            nc.scalar.activation(out=e[:], in_=lg[:], func=AF.Exp, scale=-0.5)
            a = small_pool.tile([P, NF], bf16, tag="a")
            nc.scalar.activation(
                out=a[:], in_=dot[:].rearrange("p c q -> p (c q)"), func=AF.Abs
            )
            # t = |dot| * e on Pool
            t = small_pool.tile([P, NF], bf16, tag="t")
            nc.gpsimd.tensor_mul(out=t[:], in0=a[:], in1=e[:])

            for k, (poff, w, ps) in enumerate(psums):
                if NF <= poff:
                    continue
                ww = min(w, NF - poff)
                nc.tensor.matmul(
                    out=ps[:, 0:ww],
                    lhsT=ones[:],
                    rhs=t[:, poff : poff + ww],
                    start=False,
                    stop=(last_user[k] == i),
                    skip_group_check=True,
                )

        for i in range(len(tiles)):
            if i >= 1:
                emit_b(i - 1)
            emit_a(i)
        emit_b(len(tiles) - 1)

        # Tail: reduce each PSUM bank directly (DVE reads PSUM), then the
        # tiny per-bank sums, then DMA the scalar out
        t3 = const_pool.tile([1, len(psums)], f32)
        for k, (poff, w, ps) in enumerate(psums):
            nc.vector.tensor_reduce(
                out=t3[:, k : k + 1],
                in_=ps[:],
                op=mybir.AluOpType.add,
                axis=mybir.AxisListType.X,
            )
        total = const_pool.tile([1, 1], f32)
        nc.vector.tensor_reduce(
            out=total[:], in_=t3[:], op=mybir.AluOpType.add, axis=mybir.AxisListType.X
        )
        nc.sync.dma_start(out=out_ext[:], in_=total[:])

    return nc


_NC_CACHE: dict = {}

DEFAULT_TILES = (16, 32, 48, 64, 64, 64, 64, 64, 48, 32, 16)


def _get_nc(tiles) -> bass.Bass:
    key = tuple(tiles)
    if key not in _NC_CACHE:
        nc = build_nc(list(tiles))
        _split_excess_waits(nc)
        _NC_CACHE[key] = nc
    return _NC_CACHE[key]


def kernel(jt_uvd_pred, jt_uvd_gt, _tiles=DEFAULT_TILES, _trace: bool = False):
    pred = np.ascontiguousarray(np.asarray(jt_uvd_pred), dtype=np.float32)
    gt = np.ascontiguousarray(np.asarray(jt_uvd_gt), dtype=np.float32)
    Btot = pred.shape[0]
    assert pred.shape == (Btot, J, DCOORD) and gt.shape == (Btot, J, DCOORD)
    bl = P * sum(_tiles)
    assert bl * NCORES == Btot, (Btot, _tiles)

    nc = _get_nc(_tiles)
    in_maps = []
    for c in range(NCORES):
        sl = slice(c * bl, (c + 1) * bl)
        in_maps.append(
            {
                "jt_uvd_pred": pred[sl].reshape(bl, F),
                "jt_uvd_gt": gt[sl].reshape(bl, F),
            }
        )
    res = run_bass_kernel_spmd(
        nc, in_maps, core_ids=list(range(NCORES)), trace=_trace
    )
    total = sum(float(res.results[i]["out"][0, 0]) for i in range(NCORES))
    loss = 1.0 - total / (Btot * NPAIR)
    out = np.float32(loss)
    if _trace:
        return out, res
    return out


# revision 12
# speedup vs baseline: 1.1150x; 1.1150x over previous
"""AngleLoss (HANDS17 bone-angle loss) on 8 TRN2 NeuronCores.

Math (per batch element b, bone pair (i0, i1)):
    v1 = pred[b, i0, :2] - pred[b, i1, :2]
    v2 = gt[b, i0, :2]   - gt[b, i1, :2]
    t  = |v1 . v2| / (|v1| |v2|)
    loss = mean over (b, pair) of (1 - t)

Strategy: pure data parallel over the batch; each core streams its
65536-element shard (33 MB f32) through SBUF, which pins the roofline
at the ~358 GB/s per-core DMA rate (~96 us). The elementwise pipeline
is spread over three engines so every engine's true work stays under
that floor. Measured steady-state rates driving the assignment (ns per
max-AP element): DVE 2x contiguous bf16 0.55-0.60, interleaved-read
f32->bf16 cast 0.56, bf16 uv-pair subs 0.66, stride-2 pair add 1.17;
ACT ~0.9 for everything; Pool ~3.0 (otherwise idle).

  - f32->bf16 casts from the [c, j, xy] interleaved layout (packed 8B
    uv-pair reads, the fast pattern). The pred cast is split between
    ACT and DVE for load balance; gt cast on DVE. Casting (not direct
    f32 subs) also releases the input DMA buffers early, which keeps
    the DMA queues from being paced by compute.
  - 4 bone-pair subtract gathers (DVE 2-run bf16 operands); the
    HANDS17 pair list decomposes into four affine gathers.
  - prod = v1*v2 (DVE 2x contiguous).
  - sq = dc^2 on ACT with a TRANSPOSED [xy-outer] output, so the
    |v|^2 pair-reduction is a contiguous-half add (DVE 2x).
  - dot = stride-2 pair add (DVE), den = n1*n2 (Pool), e =
    exp(-0.5 ln(den+eps)) (ACT; Rsqrt is banned in bass), a = |dot|
    (ACT), t = a*e (Pool), ones-matmul batch reduction (PE -> PSUM).

Stage B of tile i-1 is emitted BEFORE stage A of tile i so each
in-order engine queue always has ready work at its head, and the input
pool is triple-buffered so the DMA runs ~3 tiles ahead. Tile sizes
ramp small -> 64 -> small to shorten the initial DMA wait and the
final drain.
"""
import sys

sys.path.insert(0, "/opt/trn_rl_repo")

from contextlib import ExitStack

import numpy as np

import concourse.bass as bass
import concourse.tile as tile
from concourse import mybir
from concourse.bass_utils import run_bass_kernel_spmd

B, J, DCOORD = 524288, 21, 3
NCORES = 8
P = 128                      # SBUF partitions
F = J * DCOORD               # 63 floats per batch element
NPAIR = 20

f32 = mybir.dt.float32
bf16 = mybir.dt.bfloat16
AF = mybir.ActivationFunctionType


def _split_excess_waits(nc, max_waits: int = 1) -> int:
    """The staged neuronxcc rejects instructions with more than one
    semaphore wait. Same-engine instructions run in order, so excess
    waits move onto preceding NoOps on the same engine."""
    n_split = 0
    for b in nc.m.functions[0].blocks:
        insts = b.instructions
        out = []
        changed = False
        for inst in insts:
            si = getattr(inst, "sync_info", None)
            waits = list(si.on_wait) if si is not None and si.on_wait else []
            if len(waits) > max_waits:
                extra, keep = waits[:-max_waits], waits[-max_waits:]
                while extra:
                    grp, extra = extra[:max_waits], extra[max_waits:]
                    nop = mybir.InstNoOp(
                        name=f"I-waitsplit-{n_split}", engine=inst.engine
                    )
                    nop.sync_info = mybir.SyncInfo(on_wait=grp, on_update=[])
                    out.append(nop)
                    n_split += 1
                inst.sync_info = mybir.SyncInfo(
                    on_wait=keep, on_update=list(si.on_update)
                )
                changed = True
            out.append(inst)
        if changed:
            insts[:] = out
    return n_split


def build_nc(tiles) -> bass.Bass:
    """One core's kernel. `tiles` is the list of per-tile batch counts C
    (batch elements per partition); total batch = P * sum(tiles)."""
    BL = P * sum(tiles)
    nc = bass.Bass()
    x_ext = nc.declare_dram_parameter("jt_uvd_pred", [BL, F], f32, isOutput=False)
    g_ext = nc.declare_dram_parameter("jt_uvd_gt", [BL, F], f32, isOutput=False)
    out_ext = nc.declare_dram_parameter("out", [1, 1], f32, isOutput=True)
    NFMAX = NPAIR * max(tiles)

    with tile.TileContext(nc) as tc, ExitStack() as ctx:
        ins_pool = ctx.enter_context(tc.tile_pool(name="ins", bufs=3))
        mid_pool = ctx.enter_context(tc.tile_pool(name="mid", bufs=2))
        small_pool = ctx.enter_context(tc.tile_pool(name="small", bufs=2))
        const_pool = ctx.enter_context(tc.tile_pool(name="const", bufs=1))
        psum_pool = ctx.enter_context(tc.tile_pool(name="psum", bufs=1, space="PSUM"))

        ones = const_pool.tile([P, 1], bf16)
        nc.vector.memset(ones[:], 1.0)
        # bf16-rounded bones can collide -> den=0; ln(den+eps) keeps those
        # pairs at t = 0*huge = 0 instead of NaN
        eps = const_pool.tile([P, 1], f32)
        nc.vector.memset(eps[:], 1e-30)

        # PSUM accumulators for the batch reduction, <=512 f32 per bank.
        # Zeroed up front so variable-size tiles can all accumulate with
        # start=False.
        psums = []
        off = 0
        while off < NFMAX:
            w = min(512, NFMAX - off)
            ps = psum_pool.tile([1, w], f32, name=f"ps{off}", tag=f"ps{off}")
            nc.vector.memset(ps[:], 0.0)
            psums.append((off, w, ps))
            off += w
        last_user = {}
        for i, C in enumerate(tiles):
            for k, (poff, w, ps) in enumerate(psums):
                if NPAIR * C > poff:
                    last_user[k] = i

        state = {}
        b0 = 0

        def emit_a(i):
            nonlocal b0
            C = tiles[i]
            FD = C * F
            rows = P * C
            xv = x_ext[b0 : b0 + rows, :].rearrange("(p c) f -> p (c f)", p=P)
            gv = g_ext[b0 : b0 + rows, :].rearrange("(p c) f -> p (c f)", p=P)
            b0 += rows

            xt = ins_pool.tile([P, FD], f32, tag="xin")
            gt = ins_pool.tile([P, FD], f32, tag="gin")
            nc.sync.dma_start(out=xt[:], in_=xv)
            nc.sync.dma_start(out=gt[:], in_=gv)

            # combined bf16 uv tile: rows 0:C pred, C:2C gt. [c][j][xy]
            # read pattern = packed 8B runs (fast). Both casts on DVE so
            # the ACT queue never gates the subs; ins buffers free here.
            u = mid_pool.tile([P, 2 * C, J, 2], bf16, tag="u")
            xs = xt[:].rearrange("p (c j k) -> p c j k", j=J, k=DCOORD)[:, :, :, 0:2]
            gs = gt[:].rearrange("p (c j k) -> p c j k", j=J, k=DCOORD)[:, :, :, 0:2]
            nc.vector.tensor_copy(out=u[:, 0:C], in_=xs)
            nc.vector.tensor_copy(out=u[:, C : 2 * C], in_=gs)

            # bone vectors dc[(t c), q, xy]: four affine gathers, every
            # operand a packed uv pair -> DVE 2x
            dc = mid_pool.tile([P, 2 * C, NPAIR, 2], bf16, tag="dc")
            root = u[:, :, 0:1, :].broadcast_to([P, 2 * C, 5, 2])
            subs = [
                (0, root, u[:, :, 1:6, :]),
                (5, u[:, :, 1:6, :], u[:, :, 6:19:3, :]),
                (10, u[:, :, 6:19:3, :], u[:, :, 7:20:3, :]),
                (15, u[:, :, 7:20:3, :], u[:, :, 8:21:3, :]),
            ]
            for s0, in0, in1 in subs:
                nc.vector.tensor_sub(out=dc[:, :, s0 : s0 + 5, :], in0=in0, in1=in1)

            # v1*v2, contiguous bf16 -> DVE 2x
            pr = mid_pool.tile([P, C, NPAIR, 2], bf16, tag="pr")
            nc.vector.tensor_mul(
                out=pr[:].rearrange("p c q k -> p (c q k)"),
                in0=dc[:, 0:C].rearrange("p c q k -> p (c q k)"),
                in1=dc[:, C : 2 * C].rearrange("p c q k -> p (c q k)"),
            )
            # squares on ACT with TRANSPOSED [xy-outer] output so the
            # pair-reduction below is contiguous
            s = mid_pool.tile([P, 2, 2 * C, NPAIR], bf16, tag="s")
            nc.scalar.activation(
                out=s[:], in_=dc[:].rearrange("p c q k -> p k c q"), func=AF.Square
            )
            state[i] = (C, pr, s)

        def emit_b(i):
            C, pr, s = state.pop(i)
            NF = NPAIR * C
            # dot = x-part + y-part (stride-2 halves of pr)
            dot = small_pool.tile([P, C, NPAIR], bf16, tag="dot")
            nc.vector.tensor_add(out=dot[:], in0=pr[:, :, :, 0], in1=pr[:, :, :, 1])
            # n[(t c), q] = |v|^2 per tensor: contiguous halves (DVE 2x)
            n = small_pool.tile([P, 2 * C, NPAIR], bf16, tag="n")
            nc.vector.tensor_add(
                out=n[:].rearrange("p c q -> p (c q)"),
                in0=s[:, 0].rearrange("p c q -> p (c q)"),
                in1=s[:, 1].rearrange("p c q -> p (c q)"),
            )
            # den = n1 * n2 on the otherwise idle Pool engine
            den = small_pool.tile([P, C, NPAIR], bf16, tag="den")
            nc.gpsimd.tensor_mul(
                out=den[:].rearrange("p c q -> p (c q)"),
                in0=n[:, 0:C].rearrange("p c q -> p (c q)"),
                in1=n[:, C : 2 * C].rearrange("p c q -> p (c q)"),
            )
            # a = |dot| first in the ACT queue (its input is ready long
            # before den's Pool round-trip that Ln waits on)
            a = small_pool.tile([P, NF], bf16, tag="a")
            nc.scalar.activation(
                out=a[:], in_=dot[:].rearrange("p c q -> p (c q)"), func=AF.Abs
            )
            # e = 1/sqrt(den) = exp(-0.5*ln(den+eps)) on ACT (Rsqrt is
            # banned in bass for accuracy; Ln/Exp/Abs share one table set)
            lg = small_pool.tile([P, NF], bf16, tag="lg")
            nc.scalar.activation(
                out=lg[:],
                in_=den[:].rearrange("p c q -> p (c q)"),
                func=AF.Ln,
                bias=eps[:],
            )
            e = small_pool.tile([P, NF], bf16, tag="e")
            nc.scalar.activation(out=e[:], in_=lg[:], func=AF.Exp, scale=-0.5)
            # t = |dot| * e on Pool
            t = small_pool.tile([P, NF], bf16, tag="t")
            nc.gpsimd.tensor_mul(out=t[:], in0=a[:], in1=e[:])

            for k, (poff, w, ps) in enumerate(psums):
                if NF <= poff:
                    continue
                ww = min(w, NF - poff)
                nc.tensor.matmul(
                    out=ps[:, 0:ww],
                    lhsT=ones[:],
                    rhs=t[:, poff : poff + ww],
                    start=False,
                    stop=(last_user[k] == i),
                    skip_group_check=True,
                )

        for i in range(len(tiles)):
            if i >= 1:
                emit_b(i - 1)
            emit_a(i)
        emit_b(len(tiles) - 1)

        # Tail: reduce each PSUM bank directly (DVE reads PSUM), then the
        # tiny per-bank sums, then DMA the scalar out
        t3 = const_pool.tile([1, len(psums)], f32)
        for k, (poff, w, ps) in enumerate(psums):
            nc.vector.tensor_reduce(
                out=t3[:, k : k + 1],
                in_=ps[:],
                op=mybir.AluOpType.add,
                axis=mybir.AxisListType.X,
            )
        total = const_pool.tile([1, 1], f32)
        nc.vector.tensor_reduce(
            out=total[:], in_=t3[:], op=mybir.AluOpType.add, axis=mybir.AxisListType.X
        )
        nc.sync.dma_start(out=out_ext[:], in_=total[:])

    return nc


_NC_CACHE: dict = {}

DEFAULT_TILES = (16, 32, 48, 64, 64, 64, 64, 64, 48, 32, 16)


def _get_nc(tiles) -> bass.Bass:
    key = tuple(tiles)
    if key not in _NC_CACHE:
        nc = build_nc(list(tiles))
        _split_excess_waits(nc)
        _NC_CACHE[key] = nc
    return _NC_CACHE[key]


def kernel(jt_uvd_pred, jt_uvd_gt, _tiles=DEFAULT_TILES, _trace: bool = False):
    pred = np.ascontiguousarray(np.asarray(jt_uvd_pred), dtype=np.float32)
    gt = np.ascontiguousarray(np.asarray(jt_uvd_gt), dtype=np.float32)
    Btot = pred.shape[0]
    assert pred.shape == (Btot, J, DCOORD) and gt.shape == (Btot, J, DCOORD)
    bl = P * sum(_tiles)
    assert bl * NCORES == Btot, (Btot, _tiles)

    nc = _get_nc(_tiles)
    in_maps = []
    for c in range(NCORES):
        sl = slice(c * bl, (c + 1) * bl)
        in_maps.append(
            {
                "jt_uvd_pred": pred[sl].reshape(bl, F),
                "jt_uvd_gt": gt[sl].reshape(bl, F),
            }
        )
    res = run_bass_kernel_spmd(
        nc, in_maps, core_ids=list(range(NCORES)), trace=_trace
    )
    total = sum(float(res.results[i]["out"][0, 0]) for i in range(NCORES))
    loss = 1.0 - total / (Btot * NPAIR)
    out = np.float32(loss)
    if _trace:
        return out, res
    return out


# revision 13
# speedup vs baseline: 1.1484x; 1.0299x over previous
"""AngleLoss (HANDS17 bone-angle loss) on 8 TRN2 NeuronCores.

Math (per batch element b, bone pair (i0, i1)):
    v1 = pred[b, i0, :2] - pred[b, i1, :2]
    v2 = gt[b, i0, :2]   - gt[b, i1, :2]
    t  = |v1 . v2| / (|v1| |v2|)
    loss = mean over (b, pair) of (1 - t)

Strategy: pure data parallel over the batch; each core streams its
65536-element shard (33 MB f32) through SBUF, which pins the roofline
at the ~358 GB/s per-core DMA rate (~96 us). The elementwise pipeline
is spread over three engines so every engine's true work stays under
that floor. Measured steady-state rates driving the assignment (ns per
max-AP element): DVE 2x contiguous bf16 0.55-0.60, interleaved-read
f32->bf16 cast 0.56, bf16 uv-pair subs 0.66, stride-2 pair add 1.17;
ACT ~0.9 for everything; Pool ~3.0 (otherwise idle).

  - f32->bf16 casts from the [c, j, xy] interleaved layout (packed 8B
    uv-pair reads, the fast pattern). The pred cast is split between
    ACT and DVE for load balance; gt cast on DVE. Casting (not direct
    f32 subs) also releases the input DMA buffers early, which keeps
    the DMA queues from being paced by compute.
  - 4 bone-pair subtract gathers (DVE 2-run bf16 operands); the
    HANDS17 pair list decomposes into four affine gathers.
  - prod = v1*v2 (DVE 2x contiguous).
  - sq = dc^2 on ACT with a TRANSPOSED [xy-outer] output, so the
    |v|^2 pair-reduction is a contiguous-half add (DVE 2x).
  - dot = stride-2 pair add (DVE), den = n1*n2 (Pool), e =
    exp(-0.5 ln(den+eps)) (ACT; Rsqrt is banned in bass), a = |dot|
    (ACT), t = a*e (Pool), ones-matmul batch reduction (PE -> PSUM).

Stage B of tile i-1 is emitted BEFORE stage A of tile i so each
in-order engine queue always has ready work at its head, and the input
pool is triple-buffered so the DMA runs ~3 tiles ahead. Tile sizes
ramp small -> 64 -> small to shorten the initial DMA wait and the
final drain.
"""
import sys

sys.path.insert(0, "/opt/trn_rl_repo")

from contextlib import ExitStack

import numpy as np

import concourse.bass as bass
import concourse.tile as tile
from concourse import mybir
from concourse.bass_utils import run_bass_kernel_spmd

B, J, DCOORD = 524288, 21, 3
NCORES = 8
P = 128                      # SBUF partitions
F = J * DCOORD               # 63 floats per batch element
NPAIR = 20

f32 = mybir.dt.float32
bf16 = mybir.dt.bfloat16
AF = mybir.ActivationFunctionType


def _split_excess_waits(nc, max_waits: int = 1) -> int:
    """The staged neuronxcc rejects instructions with more than one
    semaphore wait. Same-engine instructions run in order, so excess
    waits move onto preceding NoOps on the same engine."""
    n_split = 0
    for b in nc.m.functions[0].blocks:
        insts = b.instructions
        out = []
        changed = False
        for inst in insts:
            si = getattr(inst, "sync_info", None)
            waits = list(si.on_wait) if si is not None and si.on_wait else []
            if len(waits) > max_waits:
                extra, keep = waits[:-max_waits], waits[-max_waits:]
                while extra:
                    grp, extra = extra[:max_waits], extra[max_waits:]
                    nop = mybir.InstNoOp(
                        name=f"I-waitsplit-{n_split}", engine=inst.engine
                    )
                    nop.sync_info = mybir.SyncInfo(on_wait=grp, on_update=[])
                    out.append(nop)
                    n_split += 1
                inst.sync_info = mybir.SyncInfo(
                    on_wait=keep, on_update=list(si.on_update)
                )
                changed = True
            out.append(inst)
        if changed:
            insts[:] = out
    return n_split


def build_nc(tiles) -> bass.Bass:
    """One core's kernel. `tiles` is the list of per-tile batch counts C
    (batch elements per partition); total batch = P * sum(tiles)."""
    BL = P * sum(tiles)
    nc = bass.Bass()
    x_ext = nc.declare_dram_parameter("jt_uvd_pred", [BL, F], f32, isOutput=False)
    g_ext = nc.declare_dram_parameter("jt_uvd_gt", [BL, F], f32, isOutput=False)
    out_ext = nc.declare_dram_parameter("out", [1, 1], f32, isOutput=True)
    NFMAX = NPAIR * max(tiles)

    with tile.TileContext(nc) as tc, ExitStack() as ctx:
        ins_pool = ctx.enter_context(tc.tile_pool(name="ins", bufs=3))
        mid_pool = ctx.enter_context(tc.tile_pool(name="mid", bufs=3))
        small_pool = ctx.enter_context(tc.tile_pool(name="small", bufs=3))
        const_pool = ctx.enter_context(tc.tile_pool(name="const", bufs=1))
        psum_pool = ctx.enter_context(tc.tile_pool(name="psum", bufs=1, space="PSUM"))

        ones = const_pool.tile([P, 1], bf16)
        nc.vector.memset(ones[:], 1.0)
        # bf16-rounded bones can collide -> den=0; ln(den+eps) keeps those
        # pairs at t = 0*huge = 0 instead of NaN
        eps = const_pool.tile([P, 1], f32)
        nc.vector.memset(eps[:], 1e-30)

        # PSUM accumulators for the batch reduction, <=512 f32 per bank.
        # Zeroed up front so variable-size tiles can all accumulate with
        # start=False.
        psums = []
        off = 0
        while off < NFMAX:
            w = min(512, NFMAX - off)
            ps = psum_pool.tile([1, w], f32, name=f"ps{off}", tag=f"ps{off}")
            nc.vector.memset(ps[:], 0.0)
            psums.append((off, w, ps))
            off += w
        last_user = {}
        for i, C in enumerate(tiles):
            for k, (poff, w, ps) in enumerate(psums):
                if NPAIR * C > poff:
                    last_user[k] = i

        state = {}
        b0 = 0

        def emit_a(i):
            nonlocal b0
            C = tiles[i]
            FD = C * F
            rows = P * C
            xv = x_ext[b0 : b0 + rows, :].rearrange("(p c) f -> p (c f)", p=P)
            gv = g_ext[b0 : b0 + rows, :].rearrange("(p c) f -> p (c f)", p=P)
            b0 += rows

            xt = ins_pool.tile([P, FD], f32, tag="xin")
            gt = ins_pool.tile([P, FD], f32, tag="gin")
            nc.sync.dma_start(out=xt[:], in_=xv)
            nc.sync.dma_start(out=gt[:], in_=gv)

            # combined bf16 uv tile: rows 0:C pred, C:2C gt. [c][j][xy]
            # read pattern = packed 8B runs (fast). Both casts on DVE so
            # the ACT queue never gates the subs; ins buffers free here.
            u = mid_pool.tile([P, 2 * C, J, 2], bf16, tag="u")
            xs = xt[:].rearrange("p (c j k) -> p c j k", j=J, k=DCOORD)[:, :, :, 0:2]
            gs = gt[:].rearrange("p (c j k) -> p c j k", j=J, k=DCOORD)[:, :, :, 0:2]
            nc.vector.tensor_copy(out=u[:, 0:C], in_=xs)
            nc.vector.tensor_copy(out=u[:, C : 2 * C], in_=gs)

            # bone vectors dc[(t c), q, xy]: four affine gathers, every
            # operand a packed uv pair -> DVE 2x
            dc = mid_pool.tile([P, 2 * C, NPAIR, 2], bf16, tag="dc")
            root = u[:, :, 0:1, :].broadcast_to([P, 2 * C, 5, 2])
            subs = [
                (0, root, u[:, :, 1:6, :]),
                (5, u[:, :, 1:6, :], u[:, :, 6:19:3, :]),
                (10, u[:, :, 6:19:3, :], u[:, :, 7:20:3, :]),
                (15, u[:, :, 7:20:3, :], u[:, :, 8:21:3, :]),
            ]
            for s0, in0, in1 in subs:
                nc.vector.tensor_sub(out=dc[:, :, s0 : s0 + 5, :], in0=in0, in1=in1)

            # v1*v2, contiguous bf16 -> DVE 2x
            pr = mid_pool.tile([P, C, NPAIR, 2], bf16, tag="pr")
            nc.vector.tensor_mul(
                out=pr[:].rearrange("p c q k -> p (c q k)"),
                in0=dc[:, 0:C].rearrange("p c q k -> p (c q k)"),
                in1=dc[:, C : 2 * C].rearrange("p c q k -> p (c q k)"),
            )
            # squares on ACT with TRANSPOSED [xy-outer] output so the
            # pair-reduction below is contiguous
            s = mid_pool.tile([P, 2, 2 * C, NPAIR], bf16, tag="s")
            nc.scalar.activation(
                out=s[:], in_=dc[:].rearrange("p c q k -> p k c q"), func=AF.Square
            )
            state[i] = (C, pr, s)

        def emit_b(i):
            C, pr, s = state.pop(i)
            NF = NPAIR * C
            # dot = x-part + y-part (stride-2 halves of pr)
            dot = small_pool.tile([P, C, NPAIR], bf16, tag="dot")
            nc.vector.tensor_add(out=dot[:], in0=pr[:, :, :, 0], in1=pr[:, :, :, 1])
            # n[(t c), q] = |v|^2 per tensor: contiguous halves (DVE 2x)
            n = small_pool.tile([P, 2 * C, NPAIR], bf16, tag="n")
            nc.vector.tensor_add(
                out=n[:].rearrange("p c q -> p (c q)"),
                in0=s[:, 0].rearrange("p c q -> p (c q)"),
                in1=s[:, 1].rearrange("p c q -> p (c q)"),
            )
            # den = n1 * n2 on the otherwise idle Pool engine
            den = small_pool.tile([P, C, NPAIR], bf16, tag="den")
            nc.gpsimd.tensor_mul(
                out=den[:].rearrange("p c q -> p (c q)"),
                in0=n[:, 0:C].rearrange("p c q -> p (c q)"),
                in1=n[:, C : 2 * C].rearrange("p c q -> p (c q)"),
            )
            # a = |dot| first in the ACT queue (its input is ready long
            # before den's Pool round-trip that Ln waits on)
            a = small_pool.tile([P, NF], bf16, tag="a")
            nc.scalar.activation(
                out=a[:], in_=dot[:].rearrange("p c q -> p (c q)"), func=AF.Abs
            )
            # e = 1/sqrt(den) = exp(-0.5*ln(den+eps)) on ACT (Rsqrt is
            # banned in bass for accuracy; Ln/Exp/Abs share one table set)
            lg = small_pool.tile([P, NF], bf16, tag="lg")
            nc.scalar.activation(
                out=lg[:],
                in_=den[:].rearrange("p c q -> p (c q)"),
                func=AF.Ln,
                bias=eps[:],
            )
            e = small_pool.tile([P, NF], bf16, tag="e")
            nc.scalar.activation(out=e[:], in_=lg[:], func=AF.Exp, scale=-0.5)
            # t = |dot| * e on Pool
            t = small_pool.tile([P, NF], bf16, tag="t")
            nc.gpsimd.tensor_mul(out=t[:], in0=a[:], in1=e[:])

            for k, (poff, w, ps) in enumerate(psums):
                if NF <= poff:
                    continue
                ww = min(w, NF - poff)
                nc.tensor.matmul(
                    out=ps[:, 0:ww],
                    lhsT=ones[:],
                    rhs=t[:, poff : poff + ww],
                    start=False,
                    stop=(last_user[k] == i),
                    skip_group_check=True,
                )

        for i in range(len(tiles)):
            if i >= 1:
                emit_b(i - 1)
            emit_a(i)
        emit_b(len(tiles) - 1)

        # Tail: reduce each PSUM bank directly (DVE reads PSUM), then the
        # tiny per-bank sums, then DMA the scalar out
        t3 = const_pool.tile([1, len(psums)], f32)
        for k, (poff, w, ps) in enumerate(psums):
            nc.vector.tensor_reduce(
                out=t3[:, k : k + 1],
                in_=ps[:],
                op=mybir.AluOpType.add,
                axis=mybir.AxisListType.X,
            )
        total = const_pool.tile([1, 1], f32)
        nc.vector.tensor_reduce(
            out=total[:], in_=t3[:], op=mybir.AluOpType.add, axis=mybir.AxisListType.X
        )
        nc.sync.dma_start(out=out_ext[:], in_=total[:])

    return nc


_NC_CACHE: dict = {}

DEFAULT_TILES = (16, 32, 48, 48, 48, 48, 48, 48, 48, 48, 32, 32, 16)


def _get_nc(tiles) -> bass.Bass:
    key = tuple(tiles)
    if key not in _NC_CACHE:
        nc = build_nc(list(tiles))
        _split_excess_waits(nc)
        _NC_CACHE[key] = nc
    return _NC_CACHE[key]


def kernel(jt_uvd_pred, jt_uvd_gt, _tiles=DEFAULT_TILES, _trace: bool = False):
    pred = np.ascontiguousarray(np.asarray(jt_uvd_pred), dtype=np.float32)
    gt = np.ascontiguousarray(np.asarray(jt_uvd_gt), dtype=np.float32)
    Btot = pred.shape[0]
    assert pred.shape == (Btot, J, DCOORD) and gt.shape == (Btot, J, DCOORD)
    bl = P * sum(_tiles)
    assert bl * NCORES == Btot, (Btot, _tiles)

    nc = _get_nc(_tiles)
    in_maps = []
    for c in range(NCORES):
        sl = slice(c * bl, (c + 1) * bl)
        in_maps.append(
            {
                "jt_uvd_pred": pred[sl].reshape(bl, F),
                "jt_uvd_gt": gt[sl].reshape(bl, F),
            }
        )
    res = run_bass_kernel_spmd(
        nc, in_maps, core_ids=list(range(NCORES)), trace=_trace
    )
    total = sum(float(res.results[i]["out"][0, 0]) for i in range(NCORES))
    loss = 1.0 - total / (Btot * NPAIR)
    out = np.float32(loss)
    if _trace:
        return out, res
    return out
